# revision 15
# baseline (speedup 1.0000x reference)
"""Causal self-attention (B=2, T=2048, C=1024, NH=16, HD=64) on 8 trn2 NeuronCores.

Sharding: 2 batch groups x 4 head-groups. Core g (0..7) handles batch b=g//4
and heads [4*(g%4), 4*(g%4)+4). Each core computes its 4 heads' attention and a
partial projection (row-split W_proj); the host sums 4 partials per batch.

Per-core pipeline (d-on-partitions "transposed" layouts throughout):
  - x^T [C, T] (host pre-transposed, bf16); Q^T/K^T via lhsT=W chunks,
    V natural [T, 256] via lhsT=x^T chunks.
  - RoPE in even/odd split form (host permutes W_q/W_k columns to evens|odds
    per head; scores are invariant to a shared d-permutation of q and k):
      E' = E*cos - O*sin ; O' = O*cos + E*sin
  - scores^T[j,i] per head via row-packed K=32 matmuls (4 heads concurrent in
    the PE array), exp on ACT (no max subtraction: |scores| <= ~3), causal
    masking via 0/1 mask multiplies on the two diagonal chunks per i-block.
  - y_u^T via col-packed AV matmuls (2 heads concurrent), denominators via
    col-packed M=1 ones-matmuls, normalization via K=1 broadcast matmuls.
  - proj: out[t, co] = y^T.T @ W_proj_rows, PSUM -> DRAM DMA.
"""

import numpy as np

B, T, C, NH, HD = 2, 2048, 1024, 16, 64
NCORES = 8
HPC = 4          # heads per core
IBS = 512        # i-block (query block) size
NIB = T // IBS   # 4 i-blocks
JCS = 128        # j-chunk (key chunk) size

_cache = {}


def _patch_tile_drain():
    """This walrus build can't encode multi-wait InstDrain: split the Tile
    tail drain into a chain of single-wait drains."""
    import concourse.tile as _tile
    if getattr(_tile.TileContext, "_drain_patched", False):
        return
    import bass_rust as _br
    from concourse.vector_clock import ScopedClock

    def _drain_and_barrier(self, tick_clock, wait_clock):
        nc = self.nc
        drain_inst = nc.sync.drain()
        wait_clock.add_sem_waits(
            drain_inst.ins, ScopedClock({None: tick_clock.global_clock})
        )
        si = drain_inst.ins.sync_info
        waits = list(si.on_wait or [])
        if len(waits) > 1:
            si.on_wait = waits[:1]
            for w in waits[1:]:
                extra = nc.sync.drain()
                extra.ins.sync_info = _br.SyncInfo(on_wait=[w], on_update=[])
        nc.all_engine_barrier()
        assert self.sems is not None
        popped = nc._tile_sem_poison_stack.pop()
        assert popped is self._sem_poison
        nc.clear_and_free_semaphores(list(self.sems.allocated().values()))
        nc.all_engine_barrier()

    _tile.TileContext._drain_and_barrier = _drain_and_barrier

    # This walrus also refuses >1 sem wait on ANY instruction: peel extra
    # waits onto ENGINE_NOP carriers inserted just before, same engine/bb.
    _orig_add = _tile.TileContext._add_instruction

    def _add_instruction(self, inst):
        si = getattr(inst, "sync_info", None)
        if si is not None and si.on_wait and len(si.on_wait) > 1:
            waits = list(si.on_wait)
            si.on_wait = waits[-1:]
            import concourse.mybir as _mb
            for w in waits[:-1]:
                nop = _mb.InstEventSemaphore(
                    name=self.nc.get_next_instruction_name(), ins=[], outs=[])
                nop.engine = inst.engine
                nop.sync_info = _br.SyncInfo(on_wait=[w], on_update=[])
                _orig_add(self, nop)
        _orig_add(self, inst)

    _tile.TileContext._add_instruction = _add_instruction
    _tile.TileContext._drain_patched = True


def build_nc():
    import concourse.bass as bass
    import concourse.mybir as mybir
    import concourse.tile as tile
    from contextlib import ExitStack

    _patch_tile_drain()
    dt = mybir.dt
    f32, f32r, bf16 = dt.float32, dt.float32r, dt.bfloat16
    AL = mybir.AluOpType
    Exp = mybir.ActivationFunctionType.Exp
    nc = bass.Bass()

    xt = nc.declare_dram_parameter("xt", [C, T], bf16, isOutput=False)
    wd = {}
    for name, w in (("qe", 128), ("qo", 128), ("ke", 128), ("ko", 128),
                    ("v", 256)):
        wd[name] = nc.declare_dram_parameter(f"w{name}", [C, w], bf16,
                                             isOutput=False)
    cosd = nc.declare_dram_parameter("cosd", [128, T], f32, isOutput=False)
    sind = nc.declare_dram_parameter("sind", [128, T], f32, isOutput=False)
    maskA = nc.declare_dram_parameter("maskA", [128, 512], bf16, isOutput=False)
    wp = nc.declare_dram_parameter("wp", [256, C], bf16, isOutput=False)
    out = nc.declare_dram_parameter("out", [T, C], f32, isOutput=True)

    scale = 1.0 / float(np.sqrt(HD))

    with tile.TileContext(nc) as tc, ExitStack() as ctx:
        const = ctx.enter_context(tc.tile_pool(name="const", bufs=1))

        # ---------- constant loads ----------
        cos_sb = const.tile([128, T], f32)
        nc.sync.dma_start(cos_sb[:], cosd[:])
        sin_sb = const.tile([128, T], f32)
        nc.sync.dma_start(sin_sb[:], sind[:])
        m0_sb = const.tile([128, 512], bf16)
        nc.sync.dma_start(m0_sb[:], maskA[:])
        wp_sb = const.tile([128, 2, C], bf16)
        nc.sync.dma_start(wp_sb[:], wp[:].rearrange("(cc p) co -> p cc co", p=128))
        ones_sb = const.tile([128, 64], bf16)
        nc.gpsimd.memset(ones_sb[:], 1.0)

        w_sb = {}
        for name, w in (("qe", 128), ("qo", 128), ("ke", 128), ("ko", 128),
                        ("v", 256)):
            t = const.tile([128, 8, w], bf16, tag=f"w_{name}", name=f"w_{name}")
            nc.sync.dma_start(t[:], wd[name][:].rearrange(
                "(cc p) j -> p cc j", p=128))
            w_sb[name] = t

        # rope'd Q^T/K^T halves; rows = 4 heads x 32 dims
        qke = {n: const.tile([128, T], bf16, tag=f"r_{n}", name=f"r_{n}")
               for n in ("qe", "qo", "ke", "ko")}
        v_sb = const.tile([128, 16, 256], bf16)   # V natural, t-chunks
        yab = const.tile([128, T], bf16)          # y^T heads A,B (normalized)
        ycd = const.tile([128, T], bf16)          # y^T heads C,D

        # ---------- phase B: QKV + RoPE ----------
        with tc.tile_pool(name="xtp", bufs=1) as xtp, \
             tc.tile_pool(name="qk_ps", bufs=4, space="PSUM") as qk_ps, \
             tc.tile_pool(name="v_ps", bufs=4, space="PSUM") as v_ps, \
             tc.tile_pool(name="rope_t", bufs=2) as rope_t:
            xt_sb = xtp.tile([128, 8, T], bf16)
            nc.sync.dma_start(xt_sb[:], xt[:].rearrange("(cc p) t -> p cc t",
                                                        p=128))

            for mk in ("k", "q"):
                for tb in range(4):
                    tsl = slice(tb * 512, (tb + 1) * 512)
                    pse = qk_ps.tile([128, 512], f32, tag="qkps")
                    for cc in range(8):
                        nc.tensor.matmul(
                            pse[:], lhsT=w_sb[mk + "e"][:, cc, :],
                            rhs=xt_sb[:, cc, tsl],
                            start=(cc == 0), stop=(cc == 7))
                    pso = qk_ps.tile([128, 512], f32, tag="qkps")
                    for cc in range(8):
                        nc.tensor.matmul(
                            pso[:], lhsT=w_sb[mk + "o"][:, cc, :],
                            rhs=xt_sb[:, cc, tsl],
                            start=(cc == 0), stop=(cc == 7))
                    # E' = E*cos - O*sin ; O' = O*cos + E*sin
                    a = rope_t.tile([128, 512], f32, tag="ra")
                    nc.vector.tensor_tensor(a[:], pse[:], cos_sb[:, tsl], AL.mult)
                    b = rope_t.tile([128, 512], f32, tag="rb")
                    nc.vector.tensor_tensor(b[:], pso[:], sin_sb[:, tsl], AL.mult)
                    nc.vector.tensor_tensor(qke[mk + "e"][:, tsl], a[:], b[:],
                                            AL.subtract)
                    c = rope_t.tile([128, 512], f32, tag="rc")
                    nc.vector.tensor_tensor(c[:], pso[:], cos_sb[:, tsl], AL.mult)
                    d = rope_t.tile([128, 512], f32, tag="rd")
                    nc.vector.tensor_tensor(d[:], pse[:], sin_sb[:, tsl], AL.mult)
                    nc.vector.tensor_tensor(qke[mk + "o"][:, tsl], c[:], d[:],
                                            AL.add)

            for tcx in range(16):
                vp = v_ps.tile([128, 256], f32, tag="vps")
                tchunk = slice(tcx * 128, (tcx + 1) * 128)
                for cc in range(8):
                    nc.tensor.matmul(
                        vp[:], lhsT=xt_sb[:, cc, tchunk],
                        rhs=w_sb["v"][:, cc, :],
                        start=(cc == 0), stop=(cc == 7))
                nc.scalar.copy(v_sb[:, tcx, :], vp[:])

        qe_sb, qo_sb = qke["qe"], qke["qo"]
        ke_sb, ko_sb = qke["ke"], qke["ko"]

        # ---------- phase C: attention ----------
        # sc tile [128,4,512]: one private PSUM bank per head so the per-head
        # E->O accumulation groups never share a zero region (start=True
        # clears the whole region). y: one private bank per head-pair.
        with tc.tile_pool(name="sc_ps", bufs=1, space="PSUM") as sc_ps, \
             tc.tile_pool(name="y_ps", bufs=1, space="PSUM") as y_psp, \
             tc.tile_pool(name="dn_ps", bufs=1, space="PSUM") as dn_ps, \
             tc.tile_pool(name="p_sb", bufs=2) as p_sbp, \
             tc.tile_pool(name="r_sb", bufs=2) as r_sbp, \
             tc.tile_pool(name="rbc_sb", bufs=2) as rbc_sbp:
            for ib in range(NIB):
                isl = slice(ib * IBS, (ib + 1) * IBS)
                y_ps = y_psp.tile([128, 2, 512], f32, tag="yps")
                dn = dn_ps.tile([128, 512], f32, tag="dnrb")
                nc.vector.memset(dn[:], 1.0)
                njc = 4 * ib + 4
                for jc in range(njc):
                    jsl = slice(jc * JCS, (jc + 1) * JCS)
                    sc = sc_ps.tile([128, 4, 512], f32, tag="scps")
                    for h in range(4):
                        hsl = slice(32 * h, 32 * h + 32)
                        nc.tensor.matmul(
                            sc[:, h, :],
                            lhsT=ke_sb[hsl, jsl],
                            rhs=qe_sb[hsl, isl],
                            start=True, stop=False, tile_position=(32 * h, 0))
                        nc.tensor.matmul(
                            sc[:, h, :],
                            lhsT=ko_sb[hsl, jsl],
                            rhs=qo_sb[hsl, isl],
                            start=False, stop=True, tile_position=(32 * h, 0))
                    pt = p_sbp.tile([128, 4, 512], bf16, tag="pt")
                    nc.scalar.activation(pt[:, 0:2, :], sc[:, 0:2, :], Exp,
                                         scale=scale)
                    nc.scalar.activation(pt[:, 2:4, :], sc[:, 2:4, :], Exp,
                                         scale=scale)
                    jcd = jc - 4 * ib
                    if jcd >= 0:
                        w = 128 * (jcd + 1)
                        msl = m0_sb[:, (3 - jcd) * 128:(3 - jcd) * 128 + w]
                        nc.vector.tensor_tensor(
                            pt[:, :, 0:w], pt[:, :, 0:w],
                            msl[:, None, :].broadcast_to([128, 4, w]), AL.mult)
                    for pair in range(2):
                        for k in range(2):
                            h = 2 * pair + k
                            nc.tensor.matmul(
                                y_ps[64 * k:64 * (k + 1), pair, :],
                                lhsT=v_sb[:, jc, 64 * h:64 * (h + 1)]
                                ,
                                rhs=pt[:, h, :],
                                start=(jc == 0), stop=(jc == njc - 1),
                                tile_position=(0, 64 * k),
                                skip_group_check=True)
                    for h in range(4):
                        nc.tensor.matmul(
                            dn[32 * h:32 * h + 1, :],
                            lhsT=ones_sb[:, 0:1],
                            rhs=pt[:, h, :],
                            start=(jc == 0), stop=(jc == njc - 1),
                            tile_position=(0, 32 * h),
                            skip_group_check=True)
                # r = 1/denom; broadcast over 64 partitions via K=1 matmuls
                r = r_sbp.tile([128, 512], f32, tag="r")
                nc.vector.reciprocal(r[:], dn[:])
                r16 = r_sbp.tile([128, 512], bf16, tag="r16")
                nc.vector.tensor_copy(r16[:], r[:])
                for pair, ytile in ((0, yab), (1, ycd)):
                    rb = dn_ps.tile([128, 512], f32, tag="dnrb")
                    for k in range(2):
                        h = 2 * pair + k
                        nc.tensor.matmul(
                            rb[64 * k:64 * (k + 1), :],
                            lhsT=ones_sb[32 * h:32 * h + 1, :],
                            rhs=r16[32 * h:32 * h + 1, :],
                            start=True, stop=True,
                            tile_position=(32 * h, 64 * k),
                            skip_group_check=True)
                    rbc = rbc_sbp.tile([128, 512], f32, tag="rbc")
                    nc.vector.tensor_copy(rbc[:], rb[:])
                    nc.vector.tensor_tensor(ytile[:, isl], y_ps[:, pair, :],
                                            rbc[:], AL.mult)

        # ---------- phase D: projection ----------
        with tc.tile_pool(name="pj_ps", bufs=4, space="PSUM") as pj_ps, \
             tc.tile_pool(name="pj_sb", bufs=4) as pj_sb:
            for tcx in range(16):
                tchunk = slice(tcx * 128, (tcx + 1) * 128)
                for cob in range(2):
                    cosl = slice(cob * 512, (cob + 1) * 512)
                    pp = pj_ps.tile([128, 512], f32, tag="pjps")
                    nc.tensor.matmul(
                        pp[:], lhsT=yab[:, tchunk],
                        rhs=wp_sb[:, 0, cosl],
                        start=True, stop=False)
                    nc.tensor.matmul(
                        pp[:], lhsT=ycd[:, tchunk],
                        rhs=wp_sb[:, 1, cosl],
                        start=False, stop=True)
                    ps = pj_sb.tile([128, 512], f32, tag="pjsb")
                    nc.vector.tensor_copy(ps[:], pp[:])
                    nc.sync.dma_start(out[tchunk, cosl], ps[:])
    return nc


def _host_prep(x, cos, sin, W_attn, W_proj):
    """Build the 8 per-core input maps (pure data movement / layout prep)."""
    import ml_dtypes
    bf16 = ml_dtypes.bfloat16
    x = np.asarray(x)
    cos = np.asarray(cos)
    sin = np.asarray(sin)
    W_attn = np.asarray(W_attn)
    W_proj = np.asarray(W_proj)

    cosf = np.ascontiguousarray(cos[0, 0][:, 0::2].T.astype(np.float32))  # [32,T]
    sinf = np.ascontiguousarray(sin[0, 0][:, 0::2].T.astype(np.float32))
    cosd = np.tile(cosf, (4, 1))  # [128, T]
    sind = np.tile(sinf, (4, 1))

    tri = (np.arange(128)[:, None] <= np.arange(128)[None, :]).astype(np.float32)
    zer = np.zeros((128, 384), np.float32)
    mA = np.concatenate([zer, tri], axis=1)   # [128, 512] = [z z z tri]

    ev = np.arange(0, HD, 2)
    od = np.arange(1, HD, 2)
    Wq, Wk, Wv = W_attn[:, 0:C], W_attn[:, C:2 * C], W_attn[:, 2 * C:3 * C]
    xt = [np.ascontiguousarray(x[b].T).astype(bf16) for b in range(B)]

    in_maps = []
    for g in range(NCORES):
        b, hg = g // HPC, g % HPC
        heads = [HPC * hg + i for i in range(HPC)]
        mk = lambda W, idx: np.ascontiguousarray(
            np.concatenate([W[:, 64 * h + idx] for h in heads], 1)).astype(bf16)
        in_maps.append({
            "xt": xt[b],
            "wqe": mk(Wq, ev), "wqo": mk(Wq, od),
            "wke": mk(Wk, ev), "wko": mk(Wk, od),
            "wv": mk(Wv, np.arange(HD)),
            "cosd": cosd, "sind": sind, "maskA": mA.astype(bf16),
            "wp": np.ascontiguousarray(np.concatenate(
                [W_proj[64 * h:64 * h + 64, :] for h in heads], 0)
            ).astype(bf16),
        })
    return in_maps


def _run(inputs, trace=False):
    from concourse.bass_utils import run_bass_kernel_spmd

    if "nc" not in _cache:
        _cache["nc"] = build_nc()
    nc = _cache["nc"]
    in_maps = _host_prep(**inputs)
    res = run_bass_kernel_spmd(
        nc, in_maps, core_ids=list(range(NCORES)), trace=trace)
    outp = np.stack([np.asarray(res.results[g]["out"], dtype=np.float32)
                     for g in range(NCORES)])
    full = np.stack([outp[4 * b:4 * b + 4].sum(axis=0) for b in range(B)])
    return full, res


def kernel(**inputs):
    full, _ = _run(inputs, trace=False)
    return full


# revision 16
# speedup vs baseline: 1.4124x; 1.4124x over previous
"""Causal self-attention (B=2, T=2048, C=1024, NH=16, HD=64) on 8 trn2 NeuronCores.

Sharding: 2 batch groups x 4 head-groups. Core g (0..7) handles batch b=g//4
and heads [4*(g%4), 4*(g%4)+4). Each core computes its 4 heads' attention and a
partial projection (row-split W_proj); the host sums 4 partials per batch.

Per-core pipeline (d-on-partitions "transposed" layouts throughout):
  - x^T [C, T] (host pre-transposed, bf16); Q^T/K^T via lhsT=W chunks,
    V natural [T, 256] via lhsT=x^T chunks.
  - RoPE in even/odd split form (host permutes W_q/W_k columns to evens|odds
    per head; scores are invariant to a shared d-permutation of q and k):
      E' = E*cos - O*sin ; O' = O*cos + E*sin
  - scores^T[j,i] per head via row-packed K=32 matmuls (4 heads concurrent in
    the PE array), exp on ACT (no max subtraction: |scores| <= ~3), causal
    masking via 0/1 mask multiplies on the two diagonal chunks per i-block.
  - y_u^T via col-packed AV matmuls (2 heads concurrent), denominators via
    col-packed M=1 ones-matmuls, normalization via K=1 broadcast matmuls.
  - proj: out[t, co] = y^T.T @ W_proj_rows, PSUM -> DRAM DMA.
"""

import numpy as np

B, T, C, NH, HD = 2, 2048, 1024, 16, 64
NCORES = 8
HPC = 4          # heads per core
IBS = 512        # i-block (query block) size
NIB = T // IBS   # 4 i-blocks
JCS = 128        # j-chunk (key chunk) size

_cache = {}


def _patch_tile_drain():
    """This walrus build can't encode multi-wait InstDrain: split the Tile
    tail drain into a chain of single-wait drains."""
    import concourse.tile as _tile
    if getattr(_tile.TileContext, "_drain_patched", False):
        return
    import bass_rust as _br
    from concourse.vector_clock import ScopedClock

    def _drain_and_barrier(self, tick_clock, wait_clock):
        nc = self.nc
        drain_inst = nc.sync.drain()
        wait_clock.add_sem_waits(
            drain_inst.ins, ScopedClock({None: tick_clock.global_clock})
        )
        si = drain_inst.ins.sync_info
        waits = list(si.on_wait or [])
        if len(waits) > 1:
            si.on_wait = waits[:1]
            for w in waits[1:]:
                extra = nc.sync.drain()
                extra.ins.sync_info = _br.SyncInfo(on_wait=[w], on_update=[])
        nc.all_engine_barrier()
        assert self.sems is not None
        popped = nc._tile_sem_poison_stack.pop()
        assert popped is self._sem_poison
        nc.clear_and_free_semaphores(list(self.sems.allocated().values()))
        nc.all_engine_barrier()

    _tile.TileContext._drain_and_barrier = _drain_and_barrier

    # This walrus also refuses >1 sem wait on ANY instruction: peel extra
    # waits onto ENGINE_NOP carriers inserted just before, same engine/bb.
    _orig_add = _tile.TileContext._add_instruction

    def _add_instruction(self, inst):
        si = getattr(inst, "sync_info", None)
        if si is not None and si.on_wait and len(si.on_wait) > 1:
            waits = list(si.on_wait)
            si.on_wait = waits[-1:]
            import concourse.mybir as _mb
            for w in waits[:-1]:
                nop = _mb.InstEventSemaphore(
                    name=self.nc.get_next_instruction_name(), ins=[], outs=[])
                nop.engine = inst.engine
                nop.sync_info = _br.SyncInfo(on_wait=[w], on_update=[])
                _orig_add(self, nop)
        _orig_add(self, inst)

    _tile.TileContext._add_instruction = _add_instruction
    _tile.TileContext._drain_patched = True


def build_nc():
    import concourse.bass as bass
    import concourse.mybir as mybir
    import concourse.tile as tile
    from contextlib import ExitStack

    _patch_tile_drain()
    dt = mybir.dt
    f32, f32r, bf16 = dt.float32, dt.float32r, dt.bfloat16
    AL = mybir.AluOpType
    Exp = mybir.ActivationFunctionType.Exp
    nc = bass.Bass()

    xt = nc.declare_dram_parameter("xt", [C, T], bf16, isOutput=False)
    wd = {}
    for name, w in (("qe", 128), ("qo", 128), ("ke", 128), ("ko", 128),
                    ("v", 256)):
        wd[name] = nc.declare_dram_parameter(f"w{name}", [C, w], bf16,
                                             isOutput=False)
    cosd = nc.declare_dram_parameter("cosd", [128, T], f32, isOutput=False)
    sind = nc.declare_dram_parameter("sind", [128, T], f32, isOutput=False)
    maskA = nc.declare_dram_parameter("maskA", [128, 512], bf16, isOutput=False)
    wp = nc.declare_dram_parameter("wp", [256, C], bf16, isOutput=False)
    out = nc.declare_dram_parameter("out", [T, C], f32, isOutput=True)

    scale = 1.0 / float(np.sqrt(HD))

    with tile.TileContext(nc) as tc, ExitStack() as ctx:
        const = ctx.enter_context(tc.tile_pool(name="const", bufs=1))

        # ---------- constant loads ----------
        cos_sb = const.tile([128, T], f32)
        nc.sync.dma_start(cos_sb[:], cosd[:])
        sin_sb = const.tile([128, T], f32)
        nc.sync.dma_start(sin_sb[:], sind[:])
        m0_sb = const.tile([128, 512], bf16)
        nc.sync.dma_start(m0_sb[:], maskA[:])
        wp_sb = const.tile([128, 2, C], bf16)
        nc.sync.dma_start(wp_sb[:], wp[:].rearrange("(cc p) co -> p cc co", p=128))
        ones_sb = const.tile([128, 64], bf16)
        nc.gpsimd.memset(ones_sb[:], 1.0)

        w_sb = {}
        for name, w in (("qe", 128), ("qo", 128), ("ke", 128), ("ko", 128),
                        ("v", 256)):
            t = const.tile([128, 8, w], bf16, tag=f"w_{name}", name=f"w_{name}")
            nc.sync.dma_start(t[:], wd[name][:].rearrange(
                "(cc p) j -> p cc j", p=128))
            w_sb[name] = t

        # rope'd Q^T/K^T halves; rows = 4 heads x 32 dims
        qke = {n: const.tile([128, T], bf16, tag=f"r_{n}", name=f"r_{n}")
               for n in ("qe", "qo", "ke", "ko")}
        v_sb = const.tile([128, 16, 256], bf16)   # V natural, t-chunks
        yab = const.tile([128, T], bf16)          # y^T heads A,B (normalized)
        ycd = const.tile([128, T], bf16)          # y^T heads C,D

        # ---------- phase B: QKV + RoPE ----------
        with tc.tile_pool(name="xtp", bufs=1) as xtp, \
             tc.tile_pool(name="qk_ps", bufs=4, space="PSUM") as qk_ps, \
             tc.tile_pool(name="v_ps", bufs=4, space="PSUM") as v_ps, \
             tc.tile_pool(name="rope_t", bufs=2) as rope_t:
            xt_sb = xtp.tile([128, 8, T], bf16)
            xt_r = xt[:].rearrange("(cc p) t -> p cc t", p=128)
            for cc in range(8):
                nc.sync.dma_start(xt_sb[:, cc, :], xt_r[:, cc, :])

            for mk in ("k", "q"):
                for tb in range(4):
                    tsl = slice(tb * 512, (tb + 1) * 512)
                    pse = qk_ps.tile([128, 512], f32, tag="qkps")
                    for cc in range(8):
                        nc.tensor.matmul(
                            pse[:], lhsT=w_sb[mk + "e"][:, cc, :],
                            rhs=xt_sb[:, cc, tsl],
                            start=(cc == 0), stop=(cc == 7))
                    pso = qk_ps.tile([128, 512], f32, tag="qkps")
                    for cc in range(8):
                        nc.tensor.matmul(
                            pso[:], lhsT=w_sb[mk + "o"][:, cc, :],
                            rhs=xt_sb[:, cc, tsl],
                            start=(cc == 0), stop=(cc == 7))
                    # E' = E*cos - O*sin ; O' = O*cos + E*sin
                    a = rope_t.tile([128, 512], f32, tag="ra")
                    nc.vector.tensor_tensor(a[:], pse[:], cos_sb[:, tsl], AL.mult)
                    b = rope_t.tile([128, 512], f32, tag="rb")
                    nc.vector.tensor_tensor(b[:], pso[:], sin_sb[:, tsl], AL.mult)
                    nc.vector.tensor_tensor(qke[mk + "e"][:, tsl], a[:], b[:],
                                            AL.subtract)
                    c = rope_t.tile([128, 512], f32, tag="rc")
                    nc.vector.tensor_tensor(c[:], pso[:], cos_sb[:, tsl], AL.mult)
                    d = rope_t.tile([128, 512], f32, tag="rd")
                    nc.vector.tensor_tensor(d[:], pse[:], sin_sb[:, tsl], AL.mult)
                    nc.vector.tensor_tensor(qke[mk + "o"][:, tsl], c[:], d[:],
                                            AL.add)

            for tcx in range(16):
                vp = v_ps.tile([128, 256], f32, tag="vps")
                tchunk = slice(tcx * 128, (tcx + 1) * 128)
                for cc in range(8):
                    nc.tensor.matmul(
                        vp[:], lhsT=xt_sb[:, cc, tchunk],
                        rhs=w_sb["v"][:, cc, :],
                        start=(cc == 0), stop=(cc == 7))
                nc.scalar.copy(v_sb[:, tcx, :], vp[:])

        qe_sb, qo_sb = qke["qe"], qke["qo"]
        ke_sb, ko_sb = qke["ke"], qke["ko"]

        # ---------- phase C: attention ----------
        # sc tile [128,4,512]: one private PSUM bank per head so the per-head
        # E->O accumulation groups never share a zero region (start=True
        # clears the whole region). y: one private bank per head-pair.
        with tc.tile_pool(name="sc_ps", bufs=2, space="PSUM") as sc_ps, \
             tc.tile_pool(name="y_ps", bufs=1, space="PSUM") as y_psp, \
             tc.tile_pool(name="dn_ps", bufs=1, space="PSUM") as dn_ps, \
             tc.tile_pool(name="p_sb", bufs=2) as p_sbp, \
             tc.tile_pool(name="r_sb", bufs=2) as r_sbp, \
             tc.tile_pool(name="rbc_sb", bufs=2) as rbc_sbp:
            for ib in range(NIB):
                isl = slice(ib * IBS, (ib + 1) * IBS)
                y_ps = y_psp.tile([128, 2, 512], f32, tag="yps")
                dn = dn_ps.tile([128, 512], f32, tag="dnrb")
                nc.vector.memset(dn[:], 1.0)
                njc = 4 * ib + 4
                for jc in range(njc):
                    jsl = slice(jc * JCS, (jc + 1) * JCS)
                    pt = p_sbp.tile([128, 4, 512], bf16, tag="pt")
                    for half in range(2):
                        sch = sc_ps.tile([128, 2, 512], f32, tag="scps")
                        for hh in range(2):
                            h = 2 * half + hh
                            hsl = slice(32 * h, 32 * h + 32)
                            nc.tensor.matmul(
                                sch[:, hh, :],
                                lhsT=ke_sb[hsl, jsl],
                                rhs=qe_sb[hsl, isl],
                                start=True, stop=False,
                                tile_position=(32 * h, 0))
                            nc.tensor.matmul(
                                sch[:, hh, :],
                                lhsT=ko_sb[hsl, jsl],
                                rhs=qo_sb[hsl, isl],
                                start=False, stop=True,
                                tile_position=(32 * h, 0))
                        nc.scalar.activation(pt[:, 2 * half:2 * half + 2, :],
                                             sch[:], Exp, scale=scale)
                    jcd = jc - 4 * ib
                    if jcd >= 0:
                        w = 128 * (jcd + 1)
                        msl = m0_sb[:, (3 - jcd) * 128:(3 - jcd) * 128 + w]
                        nc.vector.tensor_tensor(
                            pt[:, :, 0:w], pt[:, :, 0:w],
                            msl[:, None, :].broadcast_to([128, 4, w]), AL.mult)
                    for pair in range(2):
                        for k in range(2):
                            h = 2 * pair + k
                            nc.tensor.matmul(
                                y_ps[64 * k:64 * (k + 1), pair, :],
                                lhsT=v_sb[:, jc, 64 * h:64 * (h + 1)]
                                ,
                                rhs=pt[:, h, :],
                                start=(jc == 0), stop=(jc == njc - 1),
                                tile_position=(0, 64 * k),
                                skip_group_check=True)
                    for h in range(4):
                        nc.tensor.matmul(
                            dn[32 * h:32 * h + 1, :],
                            lhsT=ones_sb[:, 0:1],
                            rhs=pt[:, h, :],
                            start=(jc == 0), stop=(jc == njc - 1),
                            tile_position=(0, 32 * h),
                            skip_group_check=True)
                # r = 1/denom; broadcast over 64 partitions via K=1 matmuls
                r = r_sbp.tile([128, 512], f32, tag="r")
                nc.vector.reciprocal(r[:], dn[:])
                r16 = r_sbp.tile([128, 512], bf16, tag="r16")
                nc.vector.tensor_copy(r16[:], r[:])
                for pair, ytile in ((0, yab), (1, ycd)):
                    rb = dn_ps.tile([128, 512], f32, tag="dnrb")
                    for k in range(2):
                        h = 2 * pair + k
                        nc.tensor.matmul(
                            rb[64 * k:64 * (k + 1), :],
                            lhsT=ones_sb[32 * h:32 * h + 1, :],
                            rhs=r16[32 * h:32 * h + 1, :],
                            start=True, stop=True,
                            tile_position=(32 * h, 64 * k),
                            skip_group_check=True)
                    rbc = rbc_sbp.tile([128, 512], f32, tag="rbc")
                    nc.vector.tensor_copy(rbc[:], rb[:])
                    nc.vector.tensor_tensor(ytile[:, isl], y_ps[:, pair, :],
                                            rbc[:], AL.mult)

        # ---------- phase D: projection ----------
        with tc.tile_pool(name="pj_ps", bufs=4, space="PSUM") as pj_ps, \
             tc.tile_pool(name="pj_sb", bufs=4) as pj_sb:
            for tcx in range(16):
                tchunk = slice(tcx * 128, (tcx + 1) * 128)
                for cob in range(2):
                    cosl = slice(cob * 512, (cob + 1) * 512)
                    pp = pj_ps.tile([128, 512], f32, tag="pjps")
                    nc.tensor.matmul(
                        pp[:], lhsT=yab[:, tchunk],
                        rhs=wp_sb[:, 0, cosl],
                        start=True, stop=False)
                    nc.tensor.matmul(
                        pp[:], lhsT=ycd[:, tchunk],
                        rhs=wp_sb[:, 1, cosl],
                        start=False, stop=True)
                    ps = pj_sb.tile([128, 512], f32, tag="pjsb")
                    nc.vector.tensor_copy(ps[:], pp[:])
                    nc.sync.dma_start(out[tchunk, cosl], ps[:])
    return nc


def _host_prep(x, cos, sin, W_attn, W_proj):
    """Build the 8 per-core input maps (pure data movement / layout prep)."""
    import ml_dtypes
    bf16 = ml_dtypes.bfloat16
    x = np.asarray(x)
    cos = np.asarray(cos)
    sin = np.asarray(sin)
    W_attn = np.asarray(W_attn)
    W_proj = np.asarray(W_proj)

    cosf = np.ascontiguousarray(cos[0, 0][:, 0::2].T.astype(np.float32))  # [32,T]
    sinf = np.ascontiguousarray(sin[0, 0][:, 0::2].T.astype(np.float32))
    cosd = np.tile(cosf, (4, 1))  # [128, T]
    sind = np.tile(sinf, (4, 1))

    tri = (np.arange(128)[:, None] <= np.arange(128)[None, :]).astype(np.float32)
    zer = np.zeros((128, 384), np.float32)
    mA = np.concatenate([zer, tri], axis=1)   # [128, 512] = [z z z tri]

    ev = np.arange(0, HD, 2)
    od = np.arange(1, HD, 2)
    Wq, Wk, Wv = W_attn[:, 0:C], W_attn[:, C:2 * C], W_attn[:, 2 * C:3 * C]
    xt = [np.ascontiguousarray(x[b].T).astype(bf16) for b in range(B)]

    in_maps = []
    for g in range(NCORES):
        b, hg = g // HPC, g % HPC
        heads = [HPC * hg + i for i in range(HPC)]
        mk = lambda W, idx: np.ascontiguousarray(
            np.concatenate([W[:, 64 * h + idx] for h in heads], 1)).astype(bf16)
        in_maps.append({
            "xt": xt[b],
            "wqe": mk(Wq, ev), "wqo": mk(Wq, od),
            "wke": mk(Wk, ev), "wko": mk(Wk, od),
            "wv": mk(Wv, np.arange(HD)),
            "cosd": cosd, "sind": sind, "maskA": mA.astype(bf16),
            "wp": np.ascontiguousarray(np.concatenate(
                [W_proj[64 * h:64 * h + 64, :] for h in heads], 0)
            ).astype(bf16),
        })
    return in_maps


def _run(inputs, trace=False):
    from concourse.bass_utils import run_bass_kernel_spmd

    if "nc" not in _cache:
        _cache["nc"] = build_nc()
    nc = _cache["nc"]
    in_maps = _host_prep(**inputs)
    res = run_bass_kernel_spmd(
        nc, in_maps, core_ids=list(range(NCORES)), trace=trace)
    outp = np.stack([np.asarray(res.results[g]["out"], dtype=np.float32)
                     for g in range(NCORES)])
    full = np.stack([outp[4 * b:4 * b + 4].sum(axis=0) for b in range(B)])
    return full, res


def kernel(**inputs):
    full, _ = _run(inputs, trace=False)
    return full
